# revision 36
# baseline (speedup 1.0000x reference)
"""Bahdanau attention kernel for 8 Trainium2 NeuronCores.

Shapes (hardcoded): B=32, S=4096, ENC=DEC=UNITS=512.
Sharding: data-parallel over batch (4 batches per core); weights replicated.

Math notes:
  - context_vector = sum_s(softmax_w * dec) = dec * sum_s(w) = dec  (w sums to 1),
    so context = H_decoder @ Wd + bd computed in full fp32 on device.
  - score = tanh(enc + dec + be + bd) . Wc + bc; softmax is invariant to the
    constant bc, so bc is dropped. |score| <= sum|Wc| ~ 8, so single-pass
    softmax without max subtraction is numerically safe.
  - The big matmul enc^T = We^T @ H^T runs in bf16 (fp32 is 4x slower on PE).
    H tiles are transposed on the tensor engine (bf16, via identity).

Per-core pipeline, per 512-s macro tile (x8 per batch, x4 batches):
  xbar transpose-DMA loads H^T [e=128,s=512] bf16 straight from DRAM (H is
  pre-cast to bf16 on the host during sharding, halving HBM traffic)
  -> PE: encT[u=128,s=512] += We[e,u-chunk]^T H^T (bf16, N=512 moving)
  -> ACT: tanh(encT + bias[u]) -> bf16, bias = (dec+bd+be)^T per-partition
  -> PE: score[1,s=512] += Wc[u-chunk]^T tanh (lagged one u-chunk)
  -> ACT: exp + accum_out partial sums -> per batch: DVE sum/recip/scale.
  Decoder projection: bf16 path early (feeds tanh bias), fp32 path late
  (exact ctx output). Cost-model sim: ~164 us/core.
"""

import sys

for _p in ("/opt/trn_rl_repo",):
    if _p not in sys.path:
        sys.path.insert(0, _p)

import numpy as np

B, S, E, U = 32, 4096, 512, 512
NCORES = 8
BPC = B // NCORES  # batches per core = 4


def build_bass(ps_enc_bufs=4, ps_sc_bufs=4, score_lag=True, h_bufs=12,
               prologue_mode="ht_first"):
    import concourse.bass as bass
    import concourse.mybir as mybir
    from concourse.masks import make_identity
    from concourse.tile import TileContext

    f32 = mybir.dt.float32
    bf16 = mybir.dt.bfloat16
    AF = mybir.ActivationFunctionType
    AX = mybir.AxisListType

    nc = bass.Bass("TRN2", target_bir_lowering=False, debug=False,
                   num_devices=NCORES)

    h = nc.dram_tensor("h", [BPC, S, E], bf16, kind="ExternalInput").ap()
    hd = nc.dram_tensor("hd", [BPC, U], f32, kind="ExternalInput").ap()
    we = nc.dram_tensor("we", [E, U], bf16, kind="ExternalInput").ap()
    wd = nc.dram_tensor("wd", [U, U], f32, kind="ExternalInput").ap()
    wc = nc.dram_tensor("wc", [U, 1], bf16, kind="ExternalInput").ap()
    wd_bf = nc.dram_tensor("wd_bf", [U, U], bf16, kind="ExternalInput").ap()
    bd = nc.dram_tensor("bd", [BPC, U], f32, kind="ExternalInput").ap()
    be = nc.dram_tensor("be", [BPC, U], f32, kind="ExternalInput").ap()
    ctx_out = nc.dram_tensor("ctx", [BPC, U], f32, kind="ExternalOutput").ap()
    w_out = nc.dram_tensor("w", [BPC, S], f32, kind="ExternalOutput").ap()

    with TileContext(nc) as tc:
        with (
            tc.tile_pool(name="consts", bufs=1) as consts,
            tc.tile_pool(name="htsb", bufs=h_bufs) as htpool,
            tc.tile_pool(name="tanh", bufs=6) as tanhpool,
            tc.tile_pool(name="rows", bufs=2) as rowpool,
            tc.tile_pool(name="misc", bufs=2) as miscpool,
            tc.tile_pool(name="ps_enc", bufs=ps_enc_bufs, space="PSUM") as ps_enc,
            tc.tile_pool(name="ps_sc", bufs=ps_sc_bufs, space="PSUM") as ps_sc,
        ):
            def load_we():
                we_bf4 = consts.tile([128, 4, U], bf16, tag="we_bf")
                nc.sync.dma_start(we_bf4[:],
                                  we.rearrange("(c p) u -> p c u", p=128))
                return [we_bf4[:, ec, :] for ec in range(4)]

            def load_rest():
                wc_bf = consts.tile([128, 4], bf16, tag="wc")
                nc.sync.dma_start(wc_bf[:],
                                  wc.rearrange("(c p) o -> p (c o)", p=128))
                hd_sb = consts.tile([BPC, U], f32, tag="hd_sb")
                nc.sync.dma_start(hd_sb[:], hd[:, :])
                wdb = consts.tile([128, 4, U], bf16, tag="wd_b")
                nc.sync.dma_start(wdb[:],
                                  wd_bf.rearrange("(c p) u -> p c u", p=128))
                bd_b = consts.tile([BPC, U], f32, tag="bd_b")
                be_b = consts.tile([BPC, U], f32, tag="be_b")
                nc.sync.dma_start(bd_b[:], bd[:, :])
                nc.sync.dma_start(be_b[:], be[:, :])
                return wc_bf, hd_sb, wdb, bd_b, be_b

            def load_wd_f32():
                wd_f4 = consts.tile([128, 4, U], f32, tag="wd_f")
                nc.sync.dma_start(wd_f4[:],
                                  wd.rearrange("(c p) u -> p c u", p=128))
                return wd_f4

            def load_ht(b, u4):
                tiles = []
                for ec in range(4):
                    t = htpool.tile([128, 512], bf16, tag="ht_sb")
                    nc.sync.dma_start_transpose(
                        t[:], h[b, u4 * 512:(u4 + 1) * 512,
                                ec * 128:(ec + 1) * 128])
                    tiles.append(t)
                return tiles

            ht_pre = {}
            if prologue_mode == "copies_first":
                we_bf = load_we()
                wc_bf, hd_sb, wd_b4, bd_b, be_b = load_rest()
                ht_pre[(0, 0)] = load_ht(0, 0)
                wd_f4 = load_wd_f32()
            elif prologue_mode == "we_first":
                we_bf = load_we()
                ht_pre[(0, 0)] = load_ht(0, 0)
                wc_bf, hd_sb, wd_b4, bd_b, be_b = load_rest()
                wd_f4 = load_wd_f32()
            else:  # ht_first
                ht_pre[(0, 0)] = load_ht(0, 0)
                we_bf = load_we()
                wc_bf, hd_sb, wd_b4, bd_b, be_b = load_rest()
                wd_f4 = load_wd_f32()
            wd_bchunks = [wd_b4[:, dc, :] for dc in range(4)]
            wd_f = [wd_f4[:, dc, :] for dc in range(4)]

            id_f32 = consts.tile([128, 128], f32, tag="id_f32")
            make_identity(nc, id_f32)

            # ---- decoder projection: bf16 path feeds the tanh bias early
            # (host-fed bf16 Wd); fp32 path for the ctx output runs late ----
            biasT = consts.tile([128, 4 * BPC], f32, tag="biasT")
            ctx_sb = consts.tile([BPC, U], f32, tag="ctx_sb")
            hdT = consts.tile([128, 4 * BPC], f32, tag="hdT")
            hdT_bf = consts.tile([128, 4 * BPC], bf16, tag="hdT_bf")

            def emit_bias_chain():
                for dc in range(4):
                    tp = ps_sc.tile([128, BPC], f32, tag="ps_score")
                    nc.tensor.transpose(tp[:],
                                        hd_sb[:, dc * 128:(dc + 1) * 128],
                                        id_f32[:BPC, :BPC])
                    nc.vector.tensor_copy(hdT[:, dc * BPC:(dc + 1) * BPC],
                                          tp[:])
                nc.vector.tensor_copy(hdT_bf[:], hdT[:])
                dec_ps = ps_enc.tile([BPC, U], f32, tag="ps_enc")
                for dc in range(4):
                    nc.tensor.matmul(dec_ps[:],
                                     hdT_bf[:, dc * BPC:(dc + 1) * BPC],
                                     wd_bchunks[dc][:], start=(dc == 0),
                                     stop=(dc == 3))
                bias_sb = consts.tile([BPC, U], f32, tag="bias_sb")
                nc.vector.tensor_add(bias_sb[:], dec_ps[:], bd_b[:])
                nc.vector.tensor_add(bias_sb[:], bias_sb[:], be_b[:])
                # biasT[:, uc*BPC + b] = (dec+bd+be)[b, uc*128:+128]
                for uc in range(4):
                    tp = ps_sc.tile([128, BPC], f32, tag="ps_score")
                    nc.tensor.transpose(tp[:],
                                        bias_sb[:, uc * 128:(uc + 1) * 128],
                                        id_f32[:BPC, :BPC])
                    nc.vector.tensor_copy(biasT[:, uc * BPC:(uc + 1) * BPC],
                                          tp[:])

            def emit_ctx_chain():
                dec_ps = ps_enc.tile([BPC, U], f32, tag="ps_enc")
                for dc in range(4):
                    nc.tensor.matmul(dec_ps[:], hdT[:, dc * BPC:(dc + 1) * BPC],
                                     wd_f[dc][:], start=(dc == 0),
                                     stop=(dc == 3))
                nc.vector.tensor_add(ctx_sb[:], dec_ps[:], bd_b[:])
                nc.sync.dma_start(ctx_out[:, :], ctx_sb[:])

            emit_bias_chain()

            # ---- main loop ----
            for b in range(BPC):
                exp_row = rowpool.tile([1, S], f32, tag="exp_row")
                partials = rowpool.tile([1, 8], f32, tag="partials")
                for u4 in range(8):
                    s0 = u4 * 512
                    if (b, u4) in ht_pre:
                        ht_sb = ht_pre.pop((b, u4))
                    else:
                        ht_sb = load_ht(b, u4)
                    sc = ps_sc.tile([1, 512], f32, tag="ps_score")
                    ths = []
                    for uc in range(4):
                        enc = ps_enc.tile([128, 512], f32, tag="ps_enc")
                        for ec in range(4):
                            nc.tensor.matmul(
                                enc[:],
                                we_bf[ec][:, uc * 128:(uc + 1) * 128],
                                ht_sb[ec][:],
                                start=(ec == 0), stop=(ec == 3))
                        th = tanhpool.tile([128, 512], bf16, tag="tanh")
                        nc.scalar.activation(
                            th[:], enc[:], AF.Tanh,
                            bias=biasT[:, uc * BPC + b:uc * BPC + b + 1])
                        ths.append(th)
                        # lag the score matvec one u-chunk so PE never stalls
                        # on the ACT tanh of the current chunk
                        if (not score_lag) or uc >= 1:
                            v = uc - 1 if score_lag else uc
                            nc.tensor.matmul(sc[:], wc_bf[:, v:v + 1],
                                             ths[v][:],
                                             start=(v == 0), stop=(v == 3))
                    if score_lag:
                        nc.tensor.matmul(sc[:], wc_bf[:, 3:4], ths[3][:],
                                         start=False, stop=True)
                    nc.scalar.activation(
                        exp_row[:, s0:s0 + 512], sc[:], AF.Exp,
                        accum_out=partials[:, u4:u4 + 1])
                zs = miscpool.tile([1, 1], f32, tag="zs")
                nc.vector.reduce_sum(zs[:], partials[:], axis=AX.X)
                rz = miscpool.tile([1, 1], f32, tag="rz")
                nc.vector.reciprocal(rz[:], zs[:])
                w_sb = rowpool.tile([1, S], f32, tag="w_sb")
                nc.vector.tensor_scalar_mul(w_sb[:], exp_row[:], rz[:])
                nc.sync.dma_start(w_out[b:b + 1, :], w_sb[:])
                if b == 0:
                    emit_ctx_chain()
    return nc


def _reduce_waits(nc):
    """Drop semaphore waits provably implied by an instruction's other waits
    (vector-clock closure over the scheduled program), then split any waits
    still exceeding the per-opcode ISA wait-slot limit onto standalone
    InstEventSemaphore ops inserted just before the instruction on the same
    engine stream.  Tile's sem assignment is per-proc minimal but not
    transitively minimal, and walrus rejects e.g. a Matmult with 2 waits.
    """
    import concourse.mybir as mybir

    LIMITS = {"InstMatmult": 1, "InstLdweights": 1, "InstDMACopy": 1}
    DEFAULT_LIMIT = 1

    fn = nc.m.functions[0]
    for blk in fn.blocks:
        insts = blk.instructions
        # sem -> which engine increments it (only single-engine sems usable)
        sem_engines = {}
        for i in insts:
            si = i.sync_info
            if si is None:
                continue
            for u in si.on_update:
                ok = (u.sync_type == "semaphore"
                      and u.update_mode == "sem-add-imm"
                      and u.update_reg is None)
                sem_engines.setdefault(u.ant_name, set()).add(
                    str(i.engine) if ok else "<bad>")
        valid = {s for s, es in sem_engines.items()
                 if len(es) == 1 and "<bad>" not in es}

        def join(dst, src):
            for k, v in src.items():
                if dst.get(k, -1) < v:
                    dst[k] = v

        cum = {}
        events = {}   # sem -> list[(cum_after, clock_dict)]
        inherited = {}  # engine -> clock dict
        changed = []
        for i in insts:
            si = i.sync_info
            eng = str(i.engine)
            inh = inherited.setdefault(eng, {})
            waits = list(si.on_wait) if si is not None else []
            simple = all(w.sync_type == "semaphore"
                         and w.wait_mode == "sem-ge-imm"
                         and w.wait_reg is None for w in waits)
            contribs = []
            for w in waits:
                c = {w.ant_name: w.wait_value}
                ev = events.get(w.ant_name)
                if ev is not None:
                    for cum_after, clk in ev:
                        if cum_after >= w.wait_value:
                            join(c, clk)
                            break
                contribs.append(c)
            start = dict(inh)
            for c in contribs:
                join(start, c)
            if simple and len(waits) > 1:
                kept = list(range(len(waits)))
                k = 0
                while k < len(kept):
                    w = waits[kept[k]]
                    other = dict(inh)
                    for j in kept:
                        if j != kept[k]:
                            join(other, contribs[j])
                    if other.get(w.ant_name, -1) >= w.wait_value:
                        kept.pop(k)
                    else:
                        k += 1
                if len(kept) < len(waits):
                    si.on_wait = [waits[j] for j in kept]
                    i.sync_info = si
                    waits = si.on_wait
            join(inh, start)
            if si is not None:
                comp = dict(start)
                for u in si.on_update:
                    if u.ant_name in valid and u.update_mode == "sem-add-imm":
                        cum[u.ant_name] = cum.get(u.ant_name, 0) + u.update_value
                        comp[u.ant_name] = cum[u.ant_name]
                        events.setdefault(u.ant_name, []).append(
                            (cum[u.ant_name], comp))
            limit = LIMITS.get(type(i).__name__, DEFAULT_LIMIT)
            if si is not None and len(si.on_wait) > limit:
                changed.append(i)

        # split excess waits onto standalone event-semaphore waits
        for i in changed:
            si = i.sync_info
            limit = LIMITS.get(type(i).__name__, DEFAULT_LIMIT)
            extra = list(si.on_wait[:-limit]) if limit > 0 else list(si.on_wait)
            keep = list(si.on_wait[-limit:]) if limit > 0 else []
            pos = insts.index(i)
            for k, w in enumerate(extra):
                ev = mybir.InstEventSemaphore(
                    name=f"{i.name}-wsplit{k}", engine=i.engine,
                    ins=[], outs=[],
                    sync_info=mybir.SyncInfo(on_wait=[w], on_update=[]))
                insts.insert(pos, ev)
                pos += 1
            si.on_wait = keep
            i.sync_info = si


_NC_CACHE = None


def _get_nc():
    global _NC_CACHE
    if _NC_CACHE is None:
        nc = build_bass()
        _reduce_waits(nc)
        _NC_CACHE = nc
    return _NC_CACHE


def _stub_axon_hooks():
    # This container's antenv lacks axon_hooks; bass_utils imports it on the
    # trace path. Register a stub that reports "no hook" so trace degrades
    # gracefully instead of crashing.
    import types
    if "antenv.axon_hooks" not in sys.modules:
        m = types.ModuleType("antenv.axon_hooks")
        m.get_axon_ntff_profile_hook = lambda: None
        sys.modules["antenv.axon_hooks"] = m


def run_sharded(inputs, trace=False):
    import ml_dtypes
    _stub_axon_hooks()
    from concourse.bass_utils import run_bass_kernel_spmd

    He = np.asarray(inputs["H_encoder"], dtype=np.float32)
    He2 = He.astype(ml_dtypes.bfloat16)  # RTNE cast once on host
    Hd = np.ascontiguousarray(np.asarray(inputs["H_decoder"], dtype=np.float32))
    We = np.asarray(inputs["We"], dtype=np.float32).astype(ml_dtypes.bfloat16)
    Wd = np.ascontiguousarray(np.asarray(inputs["Wd"], dtype=np.float32))
    Wc = np.asarray(inputs["Wc"], dtype=np.float32).astype(ml_dtypes.bfloat16)
    Wd_bf = np.asarray(inputs["Wd"], dtype=np.float32).astype(ml_dtypes.bfloat16)
    be = np.ascontiguousarray(np.broadcast_to(
        np.asarray(inputs["be"], dtype=np.float32).reshape(1, U), (BPC, U)))
    bd = np.ascontiguousarray(np.broadcast_to(
        np.asarray(inputs["bd"], dtype=np.float32).reshape(1, U), (BPC, U)))
    bc = float(np.asarray(inputs["bc"]).reshape(-1)[0])  # drops out of softmax

    nc = _get_nc()
    in_maps = []
    for c in range(NCORES):
        sl = slice(c * BPC, (c + 1) * BPC)
        in_maps.append({
            "h": He2[sl],
            "hd": np.ascontiguousarray(Hd[sl]),
            "we": We, "wd": Wd, "wd_bf": Wd_bf, "wc": Wc,
            "bd": bd, "be": be,
        })
    res = run_bass_kernel_spmd(nc, in_maps, core_ids=list(range(NCORES)),
                               trace=trace)
    ctx = np.concatenate([r["ctx"] for r in res.results], axis=0)
    w = np.concatenate([r["w"] for r in res.results], axis=0)
    return (ctx.astype(np.float32), w[..., None].astype(np.float32)), res


def kernel(**inputs):
    out, _ = run_sharded(inputs, trace=False)
    return out
